# revision 89
# baseline (speedup 1.0000x reference)
"""Causal multi-head attention (B=2,T=2048,C=1024,H=16,Ca=64) on 8 trn2 cores.

Sharding: the 32 (batch, head) pairs are split across 8 cores - core c gets
batch b = c//4 and heads [4g, 4g+4) where g = c%4.  Each core computes its
heads' attention plus the partial output projection through its 256-row slice
of w_o; the host sums the 4 partials per batch.

Pipeline (per core), bf16 storage + fp32 PSUM, scores matmul in fp8-e4m3
DoubleRow (2x PE rate):
  - Q/K projections emit a [128=(2 groups x 64a), j, t] layout (head parity
    j within each 64-partition group, plus a zeros slot j=2) so the per-head
    fp8 scores matmul runs in DoubleRow mode - pair (head, zeros) - at
    0.5 cycles/row.
  - Scores are computed transposed (S^T[s,t]) per 512-t chunk; exp on the
    Act engine writes bf16 P^T tiles; diagonal-block triangles are zeroed
    by 0/1 mask multiplies on DVE.
  - A@V uses P^T blocks as the matmul stationary so y lands NATURAL
    [t, a] at only 65 moving rows per (t-block, s-block); the extra ones
    column of V yields the softmax denominators l in column 64.
  - normalize = per-partition reciprocal + broadcast multiply (DVE).
  - y_nat is transposed back via PE-transposes (128 rows each) for the
    output projection, whose [128,1024] psum is staged to SBUF on DVE/Act
    and DMA'd out in fp32 on the SP/Pool queues.
Chunk-major software pipeline with a filler queue: deadline-loose PE work
(next-chunk projections, transposes, deferred output projections) is issued
in ~200ns steps between score pairs so neither the PE nor the Act engine
(exp) ever stalls behind a long block of the other's dependencies; the final
chunk runs a fused per-t-block AV->normalize->transpose->oproj tail.
"""

import math
import sys

import numpy as np
import ml_dtypes

for _p in ("/opt/trn_rl_repo",):
    if _p not in sys.path:
        sys.path.insert(0, _p)

import concourse.bass as bass
from concourse import bacc
import concourse.mybir as mybir
from concourse.bass import ts
from concourse.tile import TileContext
from concourse.bass_utils import run_bass_kernel_spmd
from contextlib import ExitStack

F32 = mybir.dt.float32
BF16 = mybir.dt.bfloat16
FP8 = mybir.dt.float8e4
AF = mybir.ActivationFunctionType
DR = mybir.MatmulPerfMode.DoubleRow

B, T, C = 2, 2048, 1024
H, CA = 16, 64
SCALE = 1.0 / math.sqrt(CA)
NCORES = 8
HPC = 4          # heads per core
TB = T // 128    # 16 t-blocks of 128
TC = T // 512    # 4 t-chunks of 512
CK = C // 128    # 8 c-chunks

FP8_S = True     # fp8-e4m3 DoubleRow scores matmul
QK_DT = FP8 if FP8_S else BF16


def build_nc():
    nc = bacc.Bacc()
    xT_d = nc.declare_dram_parameter("xT", [TC, 128, CK, 512], BF16, isOutput=False)
    wq_d = nc.declare_dram_parameter("wq", [128, 2, CK, 128], BF16, isOutput=False)
    wk_d = nc.declare_dram_parameter("wk", [128, 2, CK, 128], BF16, isOutput=False)
    wv_d = nc.declare_dram_parameter("wv", [128, CK, 256], BF16, isOutput=False)
    wo_d = nc.declare_dram_parameter("wo", [128, 2, C], BF16, isOutput=False)
    mask_d = nc.declare_dram_parameter("mask", [128, 4, 512], BF16, isOutput=False)
    ident_d = nc.declare_dram_parameter("ident", [128, 128], BF16, isOutput=False)
    out_d = nc.declare_dram_parameter("out", [TB, 128, C], F32, isOutput=True)

    with TileContext(nc) as tc, ExitStack() as ctx:
        const = ctx.enter_context(tc.tile_pool(name="const", bufs=1))
        persist = ctx.enter_context(tc.tile_pool(name="persist", bufs=1))
        xp = ctx.enter_context(tc.tile_pool(name="xp", bufs=2))
        pbp = ctx.enter_context(tc.tile_pool(name="pbp", bufs=28))
        ynsbp = ctx.enter_context(tc.tile_pool(name="ynsbp", bufs=3))
        ytsbp = ctx.enter_context(tc.tile_pool(name="ytsbp", bufs=4))
        obp = ctx.enter_context(tc.tile_pool(name="obp", bufs=4))
        rbp = ctx.enter_context(tc.tile_pool(name="rbp", bufs=4))
        big = ctx.enter_context(tc.tile_pool(name="big", bufs=3, space="PSUM"))
        ynp = ctx.enter_context(tc.tile_pool(name="ynp", bufs=2, space="PSUM"))

        # ---- constant loads; j0 weight halves + x chunk 0 first (critical path)
        wq_sb = const.tile([128, 2, CK, 128], BF16, tag="wq", name="wq_sb")
        wk_sb = const.tile([128, 2, CK, 128], BF16, tag="wk", name="wk_sb")
        wv_sb = const.tile([128, CK, 256], BF16, tag="wv", name="wv_sb")
        wo_sb = const.tile([128, 2, C], BF16, tag="wo", name="wo_sb")
        mask_sb = const.tile([128, 4, 512], BF16, tag="mask", name="mask_sb")
        ident_sb = const.tile([128, 128], BF16, tag="ident", name="ident_sb")

        # persistent activations; q/k: [128=(grp, a), j(head parity; 2=zeros), t]
        qs = persist.tile([128, 3, T], QK_DT, tag="q", name="qs")
        ks = persist.tile([128, 3, T], QK_DT, tag="k", name="ks")
        v65 = persist.tile([128, HPC, TB, 65], BF16, tag="v", name="v65")

        xts = [None] * TC

        def load_x(tcn, eng4=False):
            t = xp.tile([128, CK, 512], BF16, tag="xt", name=f"xt{tcn}")
            if eng4:
                # first ck slice lands alone so the first matmul starts early
                nc.sync.dma_start(t[:, 0:1, :], xT_d[tcn, :, 0:1, :])
                nc.sync.dma_start(t[:, 1:3, :], xT_d[tcn, :, 1:3, :])
                nc.gpsimd.dma_start(t[:, 3:6, :], xT_d[tcn, :, 3:6, :])
                nc.scalar.dma_start(t[:, 6:8, :], xT_d[tcn, :, 6:8, :])
            else:
                nc.sync.dma_start(t[:, 0:4, :], xT_d[tcn, :, 0:4, :])
                nc.gpsimd.dma_start(t[:, 4:8, :], xT_d[tcn, :, 4:8, :])
            xts[tcn] = t

        nc.sync.dma_start(wq_sb[:, 0, 0:2], wq_d[:, 0, 0:2])
        nc.gpsimd.dma_start(wk_sb[:, 0], wk_d[:, 0])
        load_x(0, eng4=True)
        nc.sync.dma_start(wq_sb[:, 0, 2:CK], wq_d[:, 0, 2:CK])
        if FP8_S:
            # only chunk 0's t/s-range of the zero slots gates the first S
            nc.gpsimd.memset(ks[:, 2, 0:512], 0.0)
            nc.gpsimd.memset(qs[:, 2, 0:512], 0.0)
        nc.sync.dma_start(wq_sb[:, 1], wq_d[:, 1])
        nc.gpsimd.dma_start(wk_sb[:, 1], wk_d[:, 1])
        if FP8_S:
            nc.gpsimd.memset(qs[:, 2, 512:T], 0.0)
            nc.gpsimd.memset(ks[:, 2, 512:T], 0.0)
        nc.sync.dma_start(wv_sb[:], wv_d[:])
        nc.gpsimd.dma_start(wo_sb[:], wo_d[:])
        nc.sync.dma_start(mask_sb[:], mask_d[:])
        nc.sync.dma_start(ident_sb[:], ident_d[:])
        nc.vector.memset(v65[:, :, :, 64], 1.0)

        # ---- filler machinery ------------------------------------------------
        # Deadline-loose PE work (projections, transposes, output projection)
        # is queued as small steps and drained a few hundred ns at a time
        # between score pairs, so the Act engine (exp) never starves behind a
        # long block of non-score PE work.  Items: (est_ns, fn) or (None, mark).
        from collections import deque
        filler = deque()
        done_marks = set()

        def drain(budget=None):
            while filler:
                est, x = filler[0]
                if est is None:
                    filler.popleft()
                    done_marks.add(x)
                    continue
                if budget is not None and budget < est:
                    return
                filler.popleft()
                x()
                if budget is not None:
                    budget -= est

        def drain_until(mark):
            while mark not in done_marks:
                est, x = filler.popleft()
                if est is None:
                    done_marks.add(x)
                else:
                    x()

        def enq_proj_qk_j(tcn, j, w_sb, dst):
            box = {}
            for ck in range(CK):
                def mm(ck=ck):
                    if ck == 0:
                        box["ps"] = big.tile([128, 512], F32, tag="big",
                                             name="qkps")
                    nc.tensor.matmul(
                        box["ps"][:],
                        lhsT=w_sb[:, j, ck, :],
                        rhs=xts[tcn][:, ck, :],
                        start=(ck == 0), stop=(ck == CK - 1),
                    )
                filler.append((215, mm))
            def cp():
                nc.vector.tensor_copy(dst[:, j, ts(tcn, 512)], box["ps"][:])
            filler.append((60, cp))

        def enq_proj_v(tcn):
            box = {}
            for tb4 in range(4):
                for ck2 in range(0, CK, 2):
                    def mm(tb4=tb4, ck2=ck2):
                        if tb4 == 0 and ck2 == 0:
                            box["ps"] = big.tile([128, 4, 256], F32, tag="big",
                                                 name="vps")
                        for ck in (ck2, ck2 + 1):
                            nc.tensor.matmul(
                                box["ps"][:, tb4, :],
                                lhsT=xts[tcn][:, ck, ts(tb4, 128)],
                                rhs=wv_sb[:, ck, :],
                                start=(ck == 0), stop=(ck == CK - 1),
                            )
                    filler.append((215, mm))
            def cp():
                nc.vector.tensor_copy(
                    v65[:, :, 4 * tcn:4 * tcn + 4, 0:64],
                    box["ps"][:].rearrange("p tb (h a) -> p h tb a", h=HPC),
                )
            filler.append((60, cp))

        def attn_scores(tcn, h):
            """S^T + exp + mask for all s-blocks of (tcn, h); returns pb tiles."""
            nsb = 4 * tcn + 4
            p0, p1 = 64 * (h // 2), 64 * (h // 2) + 64
            jh = h % 2
            # final head: masks on Pool so the tail's AV chain never queues
            # behind DVE staging copies
            is_final = tcn == TC - 1 and h == HSEQ[-1]
            mask_eng = nc.gpsimd if is_final else nc.vector
            budget = 700
            pbs = []
            for sb2 in range(0, nsb, 2):
                sps = big.tile([128, 2, 512], F32, tag="big", name="sps")
                for jj in range(2):
                    sb = sb2 + jj
                    if FP8_S:
                        # DoubleRow pair = (head slot jh, zeros slot 2)
                        nc.tensor.matmul(
                            sps[:, jj, :],
                            lhsT=ks[p0:p1, jh:3:2 - jh, ts(sb, 128)],
                            rhs=qs[p0:p1, jh:3:2 - jh, ts(tcn, 512)],
                            start=True, stop=True, perf_mode=DR,
                        )
                    else:
                        nc.tensor.matmul(
                            sps[:, jj, :],
                            lhsT=ks[p0:p1, jh, ts(sb, 128)],
                            rhs=qs[p0:p1, jh, ts(tcn, 512)],
                            start=True, stop=True,
                        )
                pb = pbp.tile([128, 2, 512], BF16, tag="pb", name="pb")
                d0 = sb2 - 4 * tcn
                if d0 < 2:
                    nc.scalar.activation(pb[:], sps[:], AF.Exp, scale=SCALE)
                else:
                    # second diagonal pair: exp only the live tail of each block
                    for jj in range(2):
                        d = d0 + jj
                        nc.gpsimd.memset(pb[:, jj, 0:128 * d], 0.0)
                        nc.scalar.activation(
                            pb[:, jj, 128 * d:512], sps[:, jj, 128 * d:512],
                            AF.Exp, scale=SCALE,
                        )
                for jj in range(2):
                    d = sb2 + jj - 4 * tcn
                    if d >= 0:
                        if d0 < 2:
                            w = 128 * (d + 1)
                            mask_eng.tensor_mul(
                                pb[:, jj, 0:w], pb[:, jj, 0:w], mask_sb[:, d, 0:w])
                        else:
                            mask_eng.tensor_mul(
                                pb[:, jj, 128 * d:128 * (d + 1)],
                                pb[:, jj, 128 * d:128 * (d + 1)],
                                mask_sb[:, d, 128 * d:128 * (d + 1)])
                pbs.append(pb)
                drain(budget)
            return pbs

        def attn_av(tcn, h, pbs, ynsb):
            """A@V (y natural; sequential per-t-block accumulation groups -
            PSUM start zeroing is bank-granular) + normalize."""
            ynat = ynp.tile([128, 4, 65], F32, tag="yn", name="ynat")
            for tb4 in range(4):
                last = 4 * tcn + tb4
                for sb in range(last + 1):
                    nc.tensor.matmul(
                        ynat[:, tb4, :],
                        lhsT=pbs[sb // 2][:, sb % 2, ts(tb4, 128)],
                        rhs=v65[:, h, sb, :],
                        start=(sb == 0), stop=(sb == last),
                    )
            rb = rbp.tile([128, 4], F32, tag="rb", name="rb")
            nc.vector.reciprocal(rb[:], ynat[:, :, 64])
            nc.vector.tensor_mul(
                ynsb[:, :, 64 * h:64 * h + 64],
                ynat[:, :, 0:64],
                rb[:].unsqueeze(-1).broadcast_to((128, 4, 64)),
            )

        def enq_transpose(tcn, ynsb, yt):
            box = {}
            for tb4 in range(4):
                def mm(tb4=tb4):
                    if tb4 == 0:
                        box["ps"] = big.tile([128, 1024], F32, tag="big",
                                             name="ytps")
                        box["v"] = box["ps"].bitcast(BF16)[:, 0:1024].rearrange(
                            "p (cl tb t) -> p cl tb t", cl=2, tb=4)
                    for cl in range(2):
                        nc.tensor.transpose(
                            box["v"][:, cl, tb4, :],
                            ynsb[:, tb4, ts(cl, 128)],
                            ident_sb[:],
                        )
                filler.append((110, mm))
            def cp():
                nc.vector.tensor_copy(yt[:], box["v"][:])
            filler.append((60, cp))

        def oproj_mms(ops, yt, tb4):
            for cc in range(2):
                for cl in range(2):
                    nc.tensor.matmul(
                        ops[:, cc, :],
                        lhsT=yt[:, cl, tb4, :],
                        rhs=wo_sb[:, cl, ts(cc, 512)],
                        start=(cl == 0), stop=(cl == 1),
                    )

        def oproj_out(ops, tcn, tb4, copy_eng=None):
            ob = obp.tile([128, C], F32, tag="ob", name="ob")
            if copy_eng is nc.scalar:
                nc.scalar.activation(
                    ob[:], ops[:].rearrange("p c f -> p (c f)"), AF.Copy)
            else:
                nc.vector.tensor_copy(ob[:], ops[:].rearrange("p c f -> p (c f)"))
            eng = nc.sync if tb4 % 2 == 0 else nc.gpsimd
            eng.dma_start(out_d[4 * tcn + tb4], ob[:])

        def enq_oproj(tcn, yt, tb4, copy_eng=None):
            box = {}
            for cc in range(2):
                def mm(cc=cc):
                    if cc == 0:
                        box["ps"] = big.tile([128, 2, 512], F32, tag="big",
                                             name="ops")
                    for cl in range(2):
                        nc.tensor.matmul(
                            box["ps"][:, cc, :],
                            lhsT=yt[:, cl, tb4, :],
                            rhs=wo_sb[:, cl, ts(cc, 512)],
                            start=(cl == 0), stop=(cl == 1),
                        )
                filler.append((430, mm))
            filler.append((60, lambda: oproj_out(box["ps"], tcn, tb4, copy_eng)))

        # ---- main software pipeline -----------------------------------------
        # Head order (0,2,1,3): heads 0/2 need only the j0 slot of q/k, so
        # attention starts right after the j0 projections of a chunk; j1
        # projections, next-chunk projections and the previous chunk's output
        # projection drip in as filler between score pairs.
        HSEQ = (0, 2, 1, 3)
        # startup: chunk-0 j0 projections issued directly (critical path);
        # copies go to different engines so S isn't serialized behind both
        for w_sb, dst, ceng in ((wq_sb, qs, nc.vector), (wk_sb, ks, nc.scalar)):
            ps = big.tile([128, 512], F32, tag="big", name="qkps")
            for ck in range(CK):
                nc.tensor.matmul(
                    ps[:], lhsT=w_sb[:, 0, ck, :], rhs=xts[0][:, ck, :],
                    start=(ck == 0), stop=(ck == CK - 1),
                )
            if ceng is nc.scalar:
                nc.scalar.activation(dst[:, 0, ts(0, 512)], ps[:], AF.Copy)
            else:
                nc.vector.tensor_copy(dst[:, 0, ts(0, 512)], ps[:])

        ynsbs = [None] * TC
        yts = [None] * TC
        for tcn in range(TC):
            ynsb = ynsbp.tile([128, 4, 256], BF16, tag="yn", name=f"ynsb{tcn}")
            ynsbs[tcn] = ynsb
            if tcn > 0:
                drain_until(f"j0-{tcn}")
            enq_proj_qk_j(tcn, 1, wq_sb, qs)
            enq_proj_qk_j(tcn, 1, wk_sb, ks)
            filler.append((None, f"j1-{tcn}"))
            if tcn > 0:
                yts[tcn - 1] = ytsbp.tile([128, 2, 4, 128], BF16, tag="yt",
                                          name="yt")
                enq_transpose(tcn - 1, ynsbs[tcn - 1], yts[tcn - 1])
            else:
                enq_proj_v(0)
                filler.append((None, "v-0"))

            pend = []
            for k, h in enumerate(HSEQ):
                if k == 2:
                    drain_until(f"j1-{tcn}")
                pbs = attn_scores(tcn, h)
                pend.append((h, pbs))
                if k == 2:
                    drain_until(f"v-{tcn}")
                    attn_av(tcn, *pend.pop(0), ynsb)   # AV(h0)
                elif k == 3:
                    attn_av(tcn, *pend.pop(0), ynsb)   # AV(h2)
                    attn_av(tcn, *pend.pop(0), ynsb)   # AV(h1)
                if k == 0 and tcn < TC - 1:
                    load_x(tcn + 1)
                    enq_proj_qk_j(tcn + 1, 0, wq_sb, qs)
                    enq_proj_qk_j(tcn + 1, 0, wk_sb, ks)
                    filler.append((None, f"j0-{tcn + 1}"))
                    enq_proj_v(tcn + 1)
                    filler.append((None, f"v-{tcn + 1}"))
                elif k == 1 and tcn >= 2:
                    # output projections are deferred one extra chunk into the
                    # Act-bound phase where the PE has slack
                    for m in ([0] if tcn == 2 else [1, 2]):
                        for tb4 in range(4):
                            # the last deferred group's copies pop near the
                            # tail: put them on Act (idle after its last exps)
                            # so DVE is clear for the tail chain
                            enq_oproj(m, yts[m], tb4,
                                      copy_eng=nc.scalar
                                      if (m == 2 and tb4 == 3) else None)
                elif k == 3 and tcn == TC - 1:
                    # pre-issue cl0 transposes of the final chunk (heads 0/1
                    # normalized by now)
                    fin_ytps = big.tile([128, 1024], F32, tag="big",
                                        name="ytps")
                    fin_ytv = fin_ytps.bitcast(BF16)[:, 0:1024].rearrange(
                        "p (cl tb t) -> p cl tb t", cl=2, tb=4)
                    for tb4 in range(4):
                        nc.tensor.transpose(
                            fin_ytv[:, 0, tb4, :], ynsb[:, tb4, 0:128],
                            ident_sb[:])
            if tcn < TC - 1:
                attn_av(tcn, *pend.pop(0), ynsb)       # AV(h3)
        h_fin, pbs_fin = pend.pop(0)
        # final chunk tail: per-t-block AV -> normalize -> transpose ->
        # output projection pipeline (copies alternate DVE/Act; the last
        # block's staging and DMA are split across engines/queues)
        n, h = TC - 1, h_fin
        ynsb = ynsbs[n]
        yt = ytsbp.tile([128, 2, 4, 128], BF16, tag="yt", name="yt")
        ynat = ynp.tile([128, 4, 65], F32, tag="yn", name="ynat")
        for tb4 in range(4):
            last = 4 * n + tb4
            for sb in range(last + 1):
                nc.tensor.matmul(
                    ynat[:, tb4, :],
                    lhsT=pbs_fin[sb // 2][:, sb % 2, ts(tb4, 128)],
                    rhs=v65[:, h, sb, :],
                    start=(sb == 0), stop=(sb == last),
                )
            rbt = rbp.tile([128, 1], F32, tag="rbt", name="rbt")
            nc.vector.reciprocal(rbt[:], ynat[:, tb4, 64:65])
            nc.vector.tensor_mul(
                ynsb[:, tb4, 64 * h:64 * h + 64],
                ynat[:, tb4, 0:64],
                rbt[:].broadcast_to((128, 64)),
            )
            nc.tensor.transpose(
                fin_ytv[:, 1, tb4, :], ynsb[:, tb4, ts(1, 128)], ident_sb[:])
            if tb4 % 2:
                nc.scalar.activation(yt[:, :, tb4, :], fin_ytv[:, :, tb4, :],
                                     AF.Copy)
            else:
                nc.vector.tensor_copy(yt[:, :, tb4, :], fin_ytv[:, :, tb4, :])
        for tb4 in range(4):
            ops = big.tile([128, 2, 512], F32, tag="big", name="ops")
            oproj_mms(ops, yt, tb4)
            if tb4 < 3:
                oproj_out(ops, n, tb4,
                          copy_eng=nc.scalar if tb4 % 2 == 0 else None)
            else:
                ob = obp.tile([128, C], F32, tag="ob", name="ob")
                nc.vector.tensor_copy(ob[:, 0:512], ops[:, 0, :])
                nc.scalar.activation(ob[:, 512:1024], ops[:, 1, :], AF.Copy)
                nc.sync.dma_start(out_d[4 * n + tb4, :, 0:512], ob[:, 0:512])
                nc.gpsimd.dma_start(out_d[4 * n + tb4, :, 512:1024],
                                    ob[:, 512:1024])
        drain()

    nc.compile()
    return nc


_NC = None


def _get_nc():
    global _NC
    if _NC is None:
        _NC = build_nc()
    return _NC


def _mask_arr():
    p = np.arange(128)[:, None, None]
    d = np.arange(4)[None, :, None]
    f = np.arange(512)[None, None, :]
    return (128 * d + p <= f).astype(ml_dtypes.bfloat16)


def _bf16(a):
    return np.ascontiguousarray(np.asarray(a, np.float32).astype(ml_dtypes.bfloat16))


def make_in_maps(x, w_q, w_k, w_v, w_o):
    x = np.asarray(x, dtype=np.float32)
    w_q = np.asarray(w_q, dtype=np.float32)
    w_k = np.asarray(w_k, dtype=np.float32)
    w_v = np.asarray(w_v, dtype=np.float32)
    w_o = np.asarray(w_o, dtype=np.float32)
    mask = np.ascontiguousarray(_mask_arr())
    ident = np.eye(128, dtype=ml_dtypes.bfloat16)
    in_maps = []
    for c in range(NCORES):
        b, g = c // 4, c % 4
        hs = [4 * g + i for i in range(HPC)]
        # xT: [TC, 128, CK, 512] (p-major per chunk)
        xT = x[b].T.reshape(CK, 128, TC, 512).transpose(2, 1, 0, 3)

        def qk_layout(w):
            # [128, 2, CK, 128]: parity-j columns = heads (j, j+2); partition
            # group g holds head 2g+j's 64 a-columns
            per_j = []
            for j in range(2):
                cols = np.concatenate(
                    [w[hs[j]], w[hs[j + 2]]], axis=1)  # [C, 128]
                per_j.append(cols.reshape(CK, 128, 128).transpose(1, 0, 2))
            return np.stack(per_j, axis=1)  # [128, 2, CK, 128]

        wv_a = np.concatenate([w_v[h] for h in hs], axis=1)  # [C, 256]
        wv_a = wv_a.reshape(CK, 128, 256).transpose(1, 0, 2)
        wo_a = w_o[256 * g:256 * (g + 1)].reshape(2, 128, C).transpose(1, 0, 2)
        in_maps.append(dict(
            mask=mask,
            ident=ident,
            xT=_bf16(xT),
            wq=_bf16(qk_layout(w_q)),
            wk=_bf16(qk_layout(w_k)),
            wv=_bf16(wv_a),
            wo=_bf16(wo_a),
        ))
    return in_maps


def gather_out(results):
    acc = [np.zeros((T, C), np.float64) for _ in range(B)]
    for c in range(NCORES):
        acc[c // 4] += results[c]["out"].reshape(T, C).astype(np.float64)
    return np.stack([a.astype(np.float32) for a in acc])


def run(x, w_q, w_k, w_v, w_o, trace=False, **spmd_kwargs):
    nc = _get_nc()
    in_maps = make_in_maps(x, w_q, w_k, w_v, w_o)
    res = run_bass_kernel_spmd(nc, in_maps, list(range(NCORES)), trace=trace,
                               **spmd_kwargs)
    return gather_out(res.results), res


def kernel(x, w_q, w_k, w_v, w_o):
    out, _ = run(x, w_q, w_k, w_v, w_o)
    return out
